# revision 11
# baseline (speedup 1.0000x reference)
"""NeighborConsistencyLoss on 8 Trainium2 NeuronCores.

Math:  loss = mean_s(1 - mean_k cos(z[s], z[knn[s,k]]))
            = 1 - (1/(S*K)) * sum_{s,k} u(z[s]) . u(z[knn[s,k]])
where u(x) = x/|x| (eps in max(|a||b|, eps) never binds for randn data).

Sharding: replicate z (staged bf16), shard the S=1000 sampled centers
across 8 cores (125 each). Each core gathers its 125 center rows plus
125*32 = 4000 neighbor rows (1KB bf16 each) from z in HBM, normalizes,
and writes one scalar partial; host combines.

Gather strategy (the per-core bottleneck is SWDGE descriptor emission,
~8ns/row, NOT bandwidth):
 - Neighbor rows go through dma_gather (InstDMAGatherAnt): TIE-vectorized
   Q7 emission, one instruction per index window, spread across 4 SWDGE
   queue contexts which emit CONCURRENTLY on different Q7 core pairs.
 - dma_gather indices are int16, so rows are bucketed into 7 fixed
   windows [28672*g, 28672*g+32768); idx16 = row - 28672*g. Each window
   instruction has compile-time capacity 640 (mean 573 +- 22), padded
   with trailing -1 (skipped, no bytes moved). Rare overflow rows spill
   to a classic indirect-DMA tile (full int32 indices, capacity 128).
 - dma_gather places index i at partition i%128, block i//128, so slots
   are in window-sorted order, NOT (center, k) order. The host therefore
   ships per-block 0/1 matrices M[slot, center] and the group-sum
   becomes V[s,:] = sum_b (M_b * rno_b)^T @ blk_b on PE. rno (1/|row|)
   is folded into the mask by one ACT copy-with-scale per block
   ([128,128]), so the gathered data needs NO per-element scale pass.
 - Centers keep canonical order via one classic indirect gather.
 - Pad-risk slots (384..639 of each window) are memset to 1.0 before
   the gathers so never-written slots can't inject NaN/Inf into the
   masked matmuls (0 * NaN = NaN on PE).

Per block b: ssq_b[p] = sum_d blk_b[p,d]^2 (DVE x*x+accum, bf16 2x),
rno = 1/sqrt(ssq) batched per window (ACT sqrt + DVE recip), wm_b =
M_b * rno_b (ACT), V += wm_b^T @ blk_b (PE, bf16, f32 PSUM). Finally
r[p] = rno_c[p] * sum_d c[p,d]*V[p,d] (DVE fused) and partial =
maskv^T @ r (tiny matmul; maskv zeroes the 3 pad centers).
"""

import numpy as np

N, D, K, S = 200000, 512, 32, 1000
NCORES = 8
SPC = S // NCORES            # 125 samples per core
P = 128
NN = SPC * K                 # 4000 neighbor rows per core
WBASE = 28672                # window stride; offsets fit int16 (<32768)
NW = 7                       # ceil(N / WBASE)
CAP = 640                    # rows per window instruction (5 blocks)
BPW = CAP // P               # blocks per window
NB = NW * BPW + 1            # mask blocks: 35 window + 1 spill
QOF = [1, 2, 3, 0, 1, 2, 3]  # window -> swdge queue

_cache = {}


def _build_module():
    import concourse.bacc as bacc
    import concourse.bass as bass
    import concourse.mybir as mybir
    import concourse.tile as tile

    f32 = mybir.dt.float32
    bf16 = mybir.dt.bfloat16
    i32 = mybir.dt.int32
    i16 = mybir.dt.int16
    AF = mybir.ActivationFunctionType
    ALU = mybir.AluOpType

    nc = bacc.Bacc(None, target_bir_lowering=False, num_swdge_queues=4)
    z_t = nc.dram_tensor("z", [N, D], bf16, kind="ExternalInput")
    idx16_t = nc.dram_tensor("idx16", [P, NW * (CAP // 16)], i16,
                             kind="ExternalInput")
    idx32_t = nc.dram_tensor("idx32", [P, 3], i32, kind="ExternalInput")
    masks_t = nc.dram_tensor("masks", [P, NB * P], bf16, kind="ExternalInput")
    out_t = nc.dram_tensor("out", [1, 1], f32, kind="ExternalOutput")

    with tile.TileContext(nc) as tc:
        with (
            tc.tile_pool(name="const", bufs=1) as const,
            tc.tile_pool(name="gath", bufs=1) as gath,
            tc.tile_pool(name="scr", bufs=2) as scr,
            tc.tile_pool(name="wb", bufs=4) as wb,
            tc.tile_pool(name="ps", bufs=1, space="PSUM") as ps,
        ):
            idx16_sb = const.tile([P, NW * (CAP // 16)], i16, tag="idx16")
            nc.sync.dma_start(idx16_sb[:], idx16_t[:])
            idx32_sb = const.tile([P, 3], i32, tag="idx32")
            nc.sync.dma_start(idx32_sb[:], idx32_t[:])
            masks_sb = const.tile([P, NB * P], bf16, tag="masks")
            nc.scalar.dma_start(masks_sb[:], masks_t[:])

            wtiles = []
            for g in range(NW):
                wt = gath.tile([P, BPW * D], bf16, tag=f"win{g}")
                # guard pad-risk slots (384..639) against NaN garbage
                nc.vector.memset(wt[:, 3 * D:5 * D], 1.0)
                wtiles.append(wt)
            ctile = gath.tile([P, D], bf16, tag="ctile")
            stile = gath.tile([P, D], bf16, tag="stile")

            # dispatch order: queues 1,2,3 first (their emission runs on
            # other Q7 pairs while queue 0's instruction holds the seq)
            for g in (0, 1, 2, 3, 4, 5, 6):
                rows = min(32768, N - WBASE * g)
                nc.gpsimd.dma_gather(
                    out_ap=wtiles[g][:].rearrange("p (c e) -> p c e", e=D),
                    in_ap=z_t[WBASE * g:WBASE * g + rows],
                    idxs_ap=idx16_sb[:, g * (CAP // 16):(g + 1) * (CAP // 16)],
                    num_idxs=CAP,
                    num_idxs_reg=CAP,
                    elem_size=D,
                    single_packet=False,
                    queue_num=QOF[g],
                )
            nc.gpsimd.indirect_dma_start(
                out=ctile[:], out_offset=None, in_=z_t[:],
                in_offset=bass.IndirectOffsetOnAxis(
                    ap=idx32_sb[:, 0:1], axis=0),
            )
            nc.gpsimd.indirect_dma_start(
                out=stile[:], out_offset=None, in_=z_t[:],
                in_offset=bass.IndirectOffsetOnAxis(
                    ap=idx32_sb[:, 1:2], axis=0),
            )

            V = ps.tile([P, D], f32, tag="V")

            # phase 1: ssq per block (DVE); rno batched per window (ACT+DVE)
            for g in range(NW):
                ssq = const.tile([P, BPW], f32, tag=f"ssq{g}")
                for j in range(BPW):
                    blk = wtiles[g][:, j * D:(j + 1) * D]
                    sq = scr.tile([P, D], bf16, tag="sq")
                    nc.vector.scalar_tensor_tensor(
                        out=sq[:], in0=blk, scalar=1.0, in1=blk,
                        op0=ALU.mult, op1=ALU.mult,
                        accum_out=ssq[:, j:j + 1],
                    )
                sqr = const.tile([P, BPW], f32, tag=f"sqr{g}")
                rno = const.tile([P, BPW], f32, tag=f"rno{g}")
                nc.scalar.activation(sqr[:], ssq[:], AF.Sqrt)
                nc.vector.reciprocal(rno[:], sqr[:])

                # phase 2 for this window: weighted mask (ACT) + matmul (PE)
                for j in range(BPW):
                    b = g * BPW + j
                    wm = wb.tile([P, P], bf16, tag="wm")
                    nc.scalar.activation(
                        wm[:], masks_sb[:, b * P:(b + 1) * P], AF.Copy,
                        scale=rno[:, j:j + 1],
                    )
                    nc.tensor.matmul(
                        out=V[:], lhsT=wm[:],
                        rhs=wtiles[g][:, j * D:(j + 1) * D],
                        start=(b == 0), stop=False,
                    )

            # spill block
            ssq_s = const.tile([P, 1], f32, tag="ssqs")
            sq = scr.tile([P, D], bf16, tag="sq")
            nc.vector.scalar_tensor_tensor(
                out=sq[:], in0=stile[:], scalar=1.0, in1=stile[:],
                op0=ALU.mult, op1=ALU.mult, accum_out=ssq_s[:],
            )
            sqr_s = const.tile([P, 1], f32, tag="sqrs")
            rno_s = const.tile([P, 1], f32, tag="rnos")
            nc.scalar.activation(sqr_s[:], ssq_s[:], AF.Sqrt)
            nc.vector.reciprocal(rno_s[:], sqr_s[:])
            wm_s = wb.tile([P, P], bf16, tag="wm")
            nc.scalar.activation(
                wm_s[:], masks_sb[:, NW * BPW * P:(NW * BPW + 1) * P],
                AF.Copy, scale=rno_s[:, :1],
            )
            nc.tensor.matmul(
                out=V[:], lhsT=wm_s[:], rhs=stile[:], start=False, stop=True,
            )

            # center: ssq on ACT (DVE is the busier engine), then final dot
            ssq_c = const.tile([P, 1], f32, tag="ssqc")
            sc = scr.tile([P, D], bf16, tag="sq")
            nc.scalar.activation(sc[:], ctile[:], AF.Square,
                                 accum_out=ssq_c[:])
            sqr_c = const.tile([P, 1], f32, tag="sqrc")
            rno_c = const.tile([P, 1], f32, tag="rnoc")
            nc.scalar.activation(sqr_c[:], ssq_c[:], AF.Sqrt)
            nc.vector.reciprocal(rno_c[:], sqr_c[:])

            wscr = scr.tile([P, D], f32, tag="wscr")
            r = const.tile([P, 1], f32, tag="r")
            nc.vector.scalar_tensor_tensor(
                out=wscr[:], in0=ctile[:], scalar=rno_c[:, :1], in1=V[:],
                op0=ALU.mult, op1=ALU.mult, accum_out=r[:],
            )

            res_ps = ps.tile([1, 1], f32, tag="res")
            mask_f32 = idx32_sb[:, 2:3].bitcast(f32)
            nc.tensor.matmul(
                out=res_ps[:], lhsT=mask_f32, rhs=r[:], start=True, stop=True
            )
            res_sb = const.tile([1, 1], f32, tag="res_sb")
            nc.vector.tensor_copy(res_sb[:], res_ps[:])
            nc.sync.dma_start(out_t[:], res_sb[:])

    nc.compile()
    return nc


def _get_module():
    if "nc" not in _cache:
        _cache["nc"] = _build_module()
    return _cache["nc"]


def _make_in_maps(z, knn_neighbors, sample_indices):
    import ml_dtypes

    z = np.asarray(z, dtype=np.float32)
    knn = np.asarray(knn_neighbors).astype(np.int64)
    sample = np.asarray(sample_indices).astype(np.int64).ravel()
    assert z.shape == (N, D) and knn.shape == (N, K) and sample.shape == (S,)

    z_bf = np.ascontiguousarray(z.astype(ml_dtypes.bfloat16))
    pp = np.arange(P)
    maskv = (pp < SPC).astype(np.float32).view(np.int32)

    in_maps = []
    for c in range(NCORES):
        s_ids = np.zeros(P, dtype=np.int64)
        s_ids[:SPC] = sample[c * SPC:(c + 1) * SPC]
        nb_rows = knn[s_ids[:SPC]].ravel()            # [4000] row ids
        owner = np.repeat(np.arange(SPC), K)          # center of each row

        win = nb_rows // WBASE                        # window of each row
        idx16 = np.full((16, NW * (CAP // 16)), -1, dtype=np.int16)
        masks = np.zeros((P, NB * P), dtype=ml_dtypes.bfloat16)
        spill_rows, spill_owner = [], []
        for g in range(NW):
            sel = np.where(win == g)[0]
            if len(sel) > CAP:
                for e in sel[CAP:]:
                    spill_rows.append(nb_rows[e])
                    spill_owner.append(owner[e])
                sel = sel[:CAP]
            offs = (nb_rows[sel] - WBASE * g).astype(np.int16)
            ii = np.arange(len(sel))
            idx16[ii % 16, g * (CAP // 16) + ii // 16] = offs
            # slot i -> partition i%128, block g*BPW + i//128
            b = g * BPW + ii // P
            masks[ii % P, b * P + owner[sel]] = 1.0
        assert len(spill_rows) <= P, "spill overflow (pathological input)"

        sp_ids = np.zeros(P, dtype=np.int64)
        nsp = len(spill_rows)
        if nsp:
            sp_ids[:nsp] = np.asarray(spill_rows, dtype=np.int64)
            masks[np.arange(nsp), NW * BPW * P + np.asarray(spill_owner)] = 1.0

        idx16_full = np.tile(idx16, (8, 1))           # replicate for tx/rx Q7
        idx32 = np.zeros((P, 3), dtype=np.int32)
        idx32[:, 0] = s_ids
        idx32[:, 1] = sp_ids
        idx32[:, 2] = maskv
        in_maps.append({"z": z_bf, "idx16": idx16_full, "idx32": idx32,
                        "masks": masks})
    return in_maps


def _combine(results):
    total = sum(float(res["out"][0, 0]) for res in results)
    return np.array(1.0 - total / (S * K), dtype=np.float32)


def kernel(z, knn_neighbors, sample_indices):
    from concourse.bass_utils import run_bass_kernel_spmd

    nc = _get_module()
    in_maps = _make_in_maps(z, knn_neighbors, sample_indices)
    out = run_bass_kernel_spmd(nc, in_maps, core_ids=list(range(NCORES)))
    return _combine(out.results)


def run_profiled(z, knn_neighbors, sample_indices, **kw):
    """Dev helper: same as kernel() but returns (loss, BassKernelResults)
    with trace/profile enabled."""
    from concourse.bass_utils import run_bass_kernel_spmd

    nc = _get_module()
    in_maps = _make_in_maps(z, knn_neighbors, sample_indices)
    out = run_bass_kernel_spmd(
        nc, in_maps, core_ids=list(range(NCORES)), trace=True, **kw
    )
    return _combine(out.results), out


# revision 15
# speedup vs baseline: 1.0868x; 1.0868x over previous
"""NeighborConsistencyLoss on 8 Trainium2 NeuronCores.

Math:  loss = mean_s(1 - mean_k cos(z[s], z[knn[s,k]]))
            = 1 - (1/(S*K)) * sum_{s,k} u(z[s]) . u(z[knn[s,k]])
where u(x) = x/|x| (eps in max(|a||b|, eps) never binds for randn data).

Sharding: replicate z (staged bf16), shard the S=1000 sampled centers
across 8 cores (125 each). Each core gathers its 125 center rows plus
125*32 = 4000 neighbor rows (1KB bf16 each) from z in HBM, normalizes,
and writes one scalar partial; host combines.

Gather strategy (the per-core bottleneck is SWDGE descriptor emission,
~8ns/row, NOT bandwidth):
 - Neighbor rows go through dma_gather (InstDMAGatherAnt): TIE-vectorized
   Q7 emission, one instruction per index window, spread across 4 SWDGE
   queue contexts which emit CONCURRENTLY on different Q7 core pairs.
 - dma_gather indices are int16, so rows are bucketed into 7 fixed
   windows [28672*g, 28672*g+32768); idx16 = row - 28672*g. Each window
   instruction has compile-time capacity 640 (mean 573 +- 22), padded
   with trailing -1 (skipped, no bytes moved). Rare overflow rows spill
   to a classic indirect-DMA tile (full int32 indices, capacity 128).
 - dma_gather places index i at partition i%128, block i//128, so slots
   are in window-sorted order, NOT (center, k) order. The host therefore
   ships per-block 0/1 matrices M[slot, center] and the group-sum
   becomes V[s,:] = sum_b (M_b * rno_b)^T @ blk_b on PE. rno (1/|row|)
   is folded into the mask by one ACT copy-with-scale per block
   ([128,128]), so the gathered data needs NO per-element scale pass.
 - Centers keep canonical order via one classic indirect gather.
 - Pad-risk slots (384..639 of each window) are memset to 1.0 before
   the gathers so never-written slots can't inject NaN/Inf into the
   masked matmuls (0 * NaN = NaN on PE).

Per block b: ssq_b[p] = sum_d blk_b[p,d]^2 (DVE x*x+accum, bf16 2x),
rno = 1/sqrt(ssq) batched per window (ACT sqrt + DVE recip), wm_b =
M_b * rno_b (ACT), V += wm_b^T @ blk_b (PE, bf16, f32 PSUM). Finally
r[p] = rno_c[p] * sum_d c[p,d]*V[p,d] (DVE fused) and partial =
maskv^T @ r (tiny matmul; maskv zeroes the 3 pad centers).
"""

import numpy as np

N, D, K, S = 200000, 512, 32, 1000
NCORES = 8
SPC = S // NCORES            # 125 samples per core
P = 128
NN = SPC * K                 # 4000 neighbor rows per core
WBASE = 28672                # window stride; offsets fit int16 (<32768)
NW = 7                       # ceil(N / WBASE)
CAP = 640                    # rows per window instruction (5 blocks)
BPW = CAP // P               # blocks per window
NB = NW * BPW + 1            # mask blocks: 35 window + 1 spill
QOF = [1, 2, 3, 0, 1, 2, 3]  # window -> swdge queue

_cache = {}


def _build_module():
    import concourse.bacc as bacc
    import concourse.bass as bass
    import concourse.mybir as mybir
    import concourse.tile as tile

    f32 = mybir.dt.float32
    bf16 = mybir.dt.bfloat16
    i32 = mybir.dt.int32
    i16 = mybir.dt.int16
    AF = mybir.ActivationFunctionType
    ALU = mybir.AluOpType

    nc = bacc.Bacc(None, target_bir_lowering=False, num_swdge_queues=4)
    z_t = nc.dram_tensor("z", [N, D], bf16, kind="ExternalInput")
    idx16_t = nc.dram_tensor("idx16", [P, NW * (CAP // 16)], i16,
                             kind="ExternalInput")
    idx32_t = nc.dram_tensor("idx32", [P, 3], i32, kind="ExternalInput")
    masks_t = nc.dram_tensor("masks", [P, NB * P], bf16, kind="ExternalInput")
    out_t = nc.dram_tensor("out", [1, 1], f32, kind="ExternalOutput")

    with tile.TileContext(nc) as tc:
        with (
            tc.tile_pool(name="const", bufs=1) as const,
            tc.tile_pool(name="gath", bufs=1) as gath,
            tc.tile_pool(name="scr", bufs=2) as scr,
            tc.tile_pool(name="wb", bufs=4) as wb,
            tc.tile_pool(name="ps", bufs=1, space="PSUM") as ps,
        ):
            idx16_sb = const.tile([P, NW * (CAP // 16)], i16, tag="idx16")
            nc.sync.dma_start(idx16_sb[:], idx16_t[:])
            idx32_sb = const.tile([P, 3], i32, tag="idx32")
            nc.sync.dma_start(idx32_sb[:], idx32_t[:])
            masks_sb = const.tile([P, NB * P], bf16, tag="masks")
            nc.scalar.dma_start(masks_sb[:], masks_t[:])

            wtiles = []
            for g in range(NW):
                wt = gath.tile([P, BPW * D], bf16, tag=f"win{g}")
                wtiles.append(wt)
            ctile = gath.tile([P, D], bf16, tag="ctile")
            stile = gath.tile([P, D], bf16, tag="stile")

            # dispatch order: queues 1,2,3 first (their emission runs on
            # other Q7 pairs while queue 0's instruction holds the seq)
            for g in (0, 1, 2, 3, 4, 5, 6):
                rows = min(32768, N - WBASE * g)
                nc.gpsimd.dma_gather(
                    out_ap=wtiles[g][:].rearrange("p (c e) -> p c e", e=D),
                    in_ap=z_t[WBASE * g:WBASE * g + rows],
                    idxs_ap=idx16_sb[:, g * (CAP // 16):(g + 1) * (CAP // 16)],
                    num_idxs=CAP,
                    num_idxs_reg=CAP,
                    elem_size=D,
                    single_packet=False,
                    queue_num=QOF[g],
                )
            nc.gpsimd.indirect_dma_start(
                out=ctile[:], out_offset=None, in_=z_t[:],
                in_offset=bass.IndirectOffsetOnAxis(
                    ap=idx32_sb[:, 0:1], axis=0),
            )
            nc.gpsimd.indirect_dma_start(
                out=stile[:], out_offset=None, in_=z_t[:],
                in_offset=bass.IndirectOffsetOnAxis(
                    ap=idx32_sb[:, 1:2], axis=0),
            )

            V = ps.tile([P, D], f32, tag="V")

            # phase 1: ssq per block, alternating DVE (x*x+accum, stt) and
            # ACT (Square+accum) to split the ~23us of row-reduce work;
            # rno batched per window (ACT sqrt + DVE recip)
            for g in range(NW):
                ssq = const.tile([P, BPW], f32, tag=f"ssq{g}")
                for j in range(BPW):
                    b = g * BPW + j
                    blk = wtiles[g][:, j * D:(j + 1) * D]
                    sq = scr.tile([P, D], bf16, tag="sq")
                    if b % 2 == 0:
                        nc.vector.scalar_tensor_tensor(
                            out=sq[:], in0=blk, scalar=1.0, in1=blk,
                            op0=ALU.mult, op1=ALU.mult,
                            accum_out=ssq[:, j:j + 1],
                        )
                    else:
                        nc.scalar.activation(
                            sq[:], blk, AF.Square,
                            accum_out=ssq[:, j:j + 1],
                        )
                sqr = const.tile([P, BPW], f32, tag=f"sqr{g}")
                rno = const.tile([P, BPW], f32, tag=f"rno{g}")
                nc.scalar.activation(sqr[:], ssq[:], AF.Sqrt)
                nc.vector.reciprocal(rno[:], sqr[:])

                # phase 2: weighted mask on DVE (tensor_scalar runs in the
                # 2x 2-byte perf mode, ~2x cheaper than ACT copy) + matmul
                for j in range(BPW):
                    b = g * BPW + j
                    wm = wb.tile([P, P], bf16, tag="wm")
                    nc.vector.tensor_scalar_mul(
                        wm[:], masks_sb[:, b * P:(b + 1) * P],
                        rno[:, j:j + 1],
                    )
                    nc.tensor.matmul(
                        out=V[:], lhsT=wm[:],
                        rhs=wtiles[g][:, j * D:(j + 1) * D],
                        start=(b == 0), stop=False,
                    )

            # spill block
            ssq_s = const.tile([P, 1], f32, tag="ssqs")
            sq = scr.tile([P, D], bf16, tag="sq")
            nc.vector.scalar_tensor_tensor(
                out=sq[:], in0=stile[:], scalar=1.0, in1=stile[:],
                op0=ALU.mult, op1=ALU.mult, accum_out=ssq_s[:],
            )
            sqr_s = const.tile([P, 1], f32, tag="sqrs")
            rno_s = const.tile([P, 1], f32, tag="rnos")
            nc.scalar.activation(sqr_s[:], ssq_s[:], AF.Sqrt)
            nc.vector.reciprocal(rno_s[:], sqr_s[:])
            wm_s = wb.tile([P, P], bf16, tag="wm")
            nc.vector.tensor_scalar_mul(
                wm_s[:], masks_sb[:, NW * BPW * P:(NW * BPW + 1) * P],
                rno_s[:, :1],
            )
            nc.tensor.matmul(
                out=V[:], lhsT=wm_s[:], rhs=stile[:], start=False, stop=True,
            )

            # center: ssq on ACT (DVE is the busier engine), then final dot
            ssq_c = const.tile([P, 1], f32, tag="ssqc")
            sc = scr.tile([P, D], bf16, tag="sq")
            nc.scalar.activation(sc[:], ctile[:], AF.Square,
                                 accum_out=ssq_c[:])
            sqr_c = const.tile([P, 1], f32, tag="sqrc")
            rno_c = const.tile([P, 1], f32, tag="rnoc")
            nc.scalar.activation(sqr_c[:], ssq_c[:], AF.Sqrt)
            nc.vector.reciprocal(rno_c[:], sqr_c[:])

            wscr = scr.tile([P, D], f32, tag="wscr")
            r = const.tile([P, 1], f32, tag="r")
            nc.vector.scalar_tensor_tensor(
                out=wscr[:], in0=ctile[:], scalar=rno_c[:, :1], in1=V[:],
                op0=ALU.mult, op1=ALU.mult, accum_out=r[:],
            )

            res_ps = ps.tile([1, 1], f32, tag="res")
            mask_f32 = idx32_sb[:, 2:3].bitcast(f32)
            nc.tensor.matmul(
                out=res_ps[:], lhsT=mask_f32, rhs=r[:], start=True, stop=True
            )
            res_sb = const.tile([1, 1], f32, tag="res_sb")
            nc.vector.tensor_copy(res_sb[:], res_ps[:])
            nc.sync.dma_start(out_t[:], res_sb[:])

    nc.compile()
    return nc


def _get_module():
    if "nc" not in _cache:
        _cache["nc"] = _build_module()
    return _cache["nc"]


def _make_in_maps(z, knn_neighbors, sample_indices):
    import ml_dtypes

    z = np.asarray(z, dtype=np.float32)
    knn = np.asarray(knn_neighbors).astype(np.int64)
    sample = np.asarray(sample_indices).astype(np.int64).ravel()
    assert z.shape == (N, D) and knn.shape == (N, K) and sample.shape == (S,)

    z_bf = np.ascontiguousarray(z.astype(ml_dtypes.bfloat16))
    pp = np.arange(P)
    maskv = (pp < SPC).astype(np.float32).view(np.int32)

    in_maps = []
    for c in range(NCORES):
        s_ids = np.zeros(P, dtype=np.int64)
        s_ids[:SPC] = sample[c * SPC:(c + 1) * SPC]
        nb_rows = knn[s_ids[:SPC]].ravel()            # [4000] row ids
        owner = np.repeat(np.arange(SPC), K)          # center of each row

        win = nb_rows // WBASE                        # window of each row
        # pad unused slots with a VALID in-window offset (0): real data is
        # gathered there (no NaN risk, no memset guard needed); the mask
        # columns for pad slots stay zero.
        idx16 = np.zeros((16, NW * (CAP // 16)), dtype=np.int16)
        masks = np.zeros((P, NB * P), dtype=ml_dtypes.bfloat16)
        spill_rows, spill_owner = [], []
        for g in range(NW):
            sel = np.where(win == g)[0]
            if len(sel) > CAP:
                for e in sel[CAP:]:
                    spill_rows.append(nb_rows[e])
                    spill_owner.append(owner[e])
                sel = sel[:CAP]
            offs = (nb_rows[sel] - WBASE * g).astype(np.int16)
            ii = np.arange(len(sel))
            idx16[ii % 16, g * (CAP // 16) + ii // 16] = offs
            # slot i -> partition i%128, block g*BPW + i//128
            b = g * BPW + ii // P
            masks[ii % P, b * P + owner[sel]] = 1.0
        assert len(spill_rows) <= P, "spill overflow (pathological input)"

        sp_ids = np.zeros(P, dtype=np.int64)
        nsp = len(spill_rows)
        if nsp:
            sp_ids[:nsp] = np.asarray(spill_rows, dtype=np.int64)
            masks[np.arange(nsp), NW * BPW * P + np.asarray(spill_owner)] = 1.0

        idx16_full = np.tile(idx16, (8, 1))           # replicate for tx/rx Q7
        idx32 = np.zeros((P, 3), dtype=np.int32)
        idx32[:, 0] = s_ids
        idx32[:, 1] = sp_ids
        idx32[:, 2] = maskv
        in_maps.append({"z": z_bf, "idx16": idx16_full, "idx32": idx32,
                        "masks": masks})
    return in_maps


def _combine(results):
    total = sum(float(res["out"][0, 0]) for res in results)
    return np.array(1.0 - total / (S * K), dtype=np.float32)


def kernel(z, knn_neighbors, sample_indices):
    from concourse.bass_utils import run_bass_kernel_spmd

    nc = _get_module()
    in_maps = _make_in_maps(z, knn_neighbors, sample_indices)
    out = run_bass_kernel_spmd(nc, in_maps, core_ids=list(range(NCORES)))
    return _combine(out.results)


def run_profiled(z, knn_neighbors, sample_indices, **kw):
    """Dev helper: same as kernel() but returns (loss, BassKernelResults)
    with trace/profile enabled."""
    from concourse.bass_utils import run_bass_kernel_spmd

    nc = _get_module()
    in_maps = _make_in_maps(z, knn_neighbors, sample_indices)
    out = run_bass_kernel_spmd(
        nc, in_maps, core_ids=list(range(NCORES)), trace=True, **kw
    )
    return _combine(out.results), out
